# revision 17
# baseline (speedup 1.0000x reference)
"""CausalWanSelfAttention on 8 Trainium2 NeuronCores.

Sharding: 16 query chunks of 252 rows; core c owns chunks (c, 15-c) for
causal load balance.  Each core projects q/k/v for its own 504 rows
(bf16 matmuls, fp32 PSUM), ropes+norms them, AllGathers K (transposed)
and V (rows) across the 8 cores in bf16, runs frame-causal flash
attention for its queries over all keys, and projects the output rows.
Host assembles the full [1, 4032, 1536] output from per-core row slices.

Optimizations over the original fp32r version (1117us -> 589us):
- all matmul operands bf16: full-rate matmuls at any free width plus
  fast weight loads (FWL needs exactly 128 weight columns, so kv rows
  are stored padded 504->512 per core and k-tiles are 128 rows; pad
  rows are zeroed and masked off).
- phase A emission: pass-A(K), pass-A(V), rope/transpose pass-B(K),
  then the four half-gathers (K/V split by head halves, interleaved
  K_A V_A K_B V_B on the serial CC stream), then pass-A/B(Q) — the
  tensor queue never stalls behind vector rope work and the gathers
  overlap Q projection + early attention heads.
- attention: den/PV matmuls for block (h, qb) are emitted one block
  late, so the tensor queue always has the next block's QK in front of
  matmuls whose exp results are ready — no PE stalls on the scalar
  engine.  Softmax denominator accumulated as a [128, q] broadcast via
  a ones-matrix lhsT (full-lane vector reciprocal, no DMA roundtrip);
  exp grouped 4 k-tiles per activation; mask multiplies batched one
  per exp group.
- every PSUM matmul accumulator is bank-aligned: a matmul output must
  not cross a 2KB PSUM bank, and two interleaved accumulation groups
  must never share one (start=True clears the whole bank).
"""
import sys, math

sys.path.insert(0, "/opt/trn_rl_repo")

import numpy as np

import concourse.bass as bass
import concourse.tile as tile
from concourse import mybir
from concourse.masks import make_identity

F32 = mybir.dt.float32
BF16 = mybir.dt.bfloat16

# Static problem geometry
S, D, H, HD = 4032, 1536, 12, 128
F_FRAMES, H_GRID, W_GRID = 9, 16, 28
FRAME = H_GRID * W_GRID            # 448
ROT = HD // 2                      # 64
EPS = 1e-6
SCALE = 1.0 / math.sqrt(HD)

NC = 8                             # cores
NCHUNK = 16                        # query chunks
CH = S // NCHUNK                   # 252 rows per chunk
KT = 128                           # k-tile rows (padded kv space)
RPAD = 512                         # padded kv rows per core (2 x 256)
SPAD = NC * RPAD                   # 4096 padded kv rows
NKT = SPAD // KT                   # 32 k-tiles
ST = 126                           # q row-tile (4 per core)
NST = 4
ROWS = 2 * CH                      # 504 own rows per core
CB = 512                           # projection column block
NCB = D // CB                      # 3
NKC = D // 128                     # 12 contraction chunks
EG = 4                             # k-tiles per exp group

# chunk pair owned by core c: (c, 15-c)
def core_chunks(c):
    return (c, NCHUNK - 1 - c)

# global row range of chunk j
def chunk_rows(j):
    return CH * j, CH * (j + 1)

# k-tile kt (padded row order) -> (source core, col offset within its 512)
def kt_src(kt):
    j = kt // 2
    src = j if j < NC else NCHUNK - 1 - j
    sub = 0 if j < NC else 1
    return src, sub * 256 + (kt % 2) * KT

# kv rows are stored padded: chunk j's 252 rows sit at padded offsets
# [256j, 256j+252); offsets 252-255 of each 256-block are zero pad.
def _pad_pos(g):
    return 256 * (g // 252) + (g % 252)

# static per-qb k-loop bounds (tiles of KT in padded space), max over cores
def _bounds():
    bnd = []
    for qb in range(2):
        mx = 0
        for c in range(NC):
            j = core_chunks(c)[qb]
            last_row = CH * j + CH - 1
            keys = (last_row // FRAME + 1) * FRAME
            mx = max(mx, _pad_pos(keys - 1) // KT + 1)
        bnd.append(mx)
    return bnd

BND = _bounds()                    # [18, 32]

# minimum causal bound over cores per qb (global rows)
def _min_bound(qb):
    return min((CH * core_chunks(c)[qb] // FRAME + 1) * FRAME
               for c in range(NC))

def _tile_clean(qb, t):
    # odd tiles contain pad rows and always need a mask; even tiles are
    # clean when every real row is allowed for every core's chunk
    if t % 2 == 1:
        return False
    return 252 * (t // 2) + KT - 1 < _min_bound(qb)

# exp groups per qb: list of (g0, gn)
def _groups(qb):
    return [(g0, min(EG, BND[qb] - g0)) for g0 in range(0, BND[qb], EG)]

# mask slots, batched per exp group: a group with any masked tile gets
# contiguous slots for ALL its tiles (clean tiles get all-ones masks) so
# the whole group is one vector multiply.
MASK_GROUP = {}                    # (qb, g0) -> slot of tile g0
_slot = 0
for _qb in range(2):
    for _g0, _gn in _groups(_qb):
        if not all(_tile_clean(_qb, _g0 + _i) for _i in range(_gn)):
            MASK_GROUP[(_qb, _g0)] = _slot
            _slot += _gn
N_MASKS = _slot

DMA_HOIST_SKIP = {"InstCall", "InstNoOp"}


def fix_waits(nc):
    """This container's walrus rejects any instruction carrying more
    than one sync wait.  Hoist waits onto preceding NoOps (one wait per
    NoOp) on the same engine; sequencer program order preserves the
    semantics."""
    nop_id = [0]
    for f in nc.m.functions:
        for blk in f.blocks:
            out, changed = [], False
            for inst in blk.instructions:
                tname = type(inst).__name__
                si = inst.sync_info
                if (tname not in DMA_HOIST_SKIP and si is not None
                        and len(si.on_wait) > 1):
                    for w in list(si.on_wait):
                        nop = mybir.InstNoOp(name=f"I-waitnop-{nop_id[0]}")
                        nop_id[0] += 1
                        nop.engine = inst.engine
                        nop.sync_info = mybir.SyncInfo(on_wait=[w], on_update=[])
                        out.append(nop)
                    si.on_wait = []
                    changed = True
                out.append(inst)
            if changed:
                blk.instructions = out


def build_program():
    nc = bass.Bass(num_devices=NC)

    xT_own = nc.dram_tensor("xT_own", [D, ROWS], BF16, kind="ExternalInput")
    Wq = nc.dram_tensor("Wq", [D, D], BF16, kind="ExternalInput")
    Wk = nc.dram_tensor("Wk", [D, D], BF16, kind="ExternalInput")
    Wv = nc.dram_tensor("Wv", [D, D], BF16, kind="ExternalInput")
    Wo = nc.dram_tensor("Wo", [D, D], BF16, kind="ExternalInput")
    bq = nc.dram_tensor("bq", [D], BF16, kind="ExternalInput")
    bk = nc.dram_tensor("bk", [D], BF16, kind="ExternalInput")
    bv = nc.dram_tensor("bv", [D], BF16, kind="ExternalInput")
    bo = nc.dram_tensor("bo", [D], BF16, kind="ExternalInput")
    gq = nc.dram_tensor("gq", [D], BF16, kind="ExternalInput")
    gk = nc.dram_tensor("gk", [D], BF16, kind="ExternalInput")
    cosf = nc.dram_tensor("cosf", [ROWS, HD], BF16, kind="ExternalInput")
    sinf = nc.dram_tensor("sinf", [ROWS, HD], BF16, kind="ExternalInput")
    masks = nc.dram_tensor("masks", [max(N_MASKS, 1), KT, CH], BF16,
                           kind="ExternalInput")
    out_own = nc.dram_tensor("out_own", [ROWS, D], F32, kind="ExternalOutput")

    DH = D // 2                     # head-half feature split (heads 0-5 / 6-11)
    kT_bA = nc.dram_tensor("kT_bA", [DH, RPAD], BF16)
    kT_bB = nc.dram_tensor("kT_bB", [DH, RPAD], BF16)
    v_bA = nc.dram_tensor("v_bA", [RPAD, DH], BF16)
    v_bB = nc.dram_tensor("v_bB", [RPAD, DH], BF16)
    kT_gA = nc.dram_tensor("kT_gA", [NC, DH, RPAD], BF16, addr_space="Shared")
    kT_gB = nc.dram_tensor("kT_gB", [NC, DH, RPAD], BF16, addr_space="Shared")
    v_gA = nc.dram_tensor("v_gA", [NC, RPAD, DH], BF16, addr_space="Shared")
    v_gB = nc.dram_tensor("v_gB", [NC, RPAD, DH], BF16, addr_space="Shared")

    with tile.TileContext(nc) as tc:
        with tc.tile_pool(name="persist", bufs=1) as persist:
            # ---- constants ----
            ident = persist.tile([128, 128], F32)
            make_identity(nc, ident[:])
            ident_bf = persist.tile([128, 128], BF16)
            nc.vector.tensor_copy(ident_bf[:], ident[:])
            ones_bf = persist.tile([128, 128], BF16)
            nc.gpsimd.memset(ones_bf[:], 1.0)
            eps_c = persist.tile([128, 1], F32)
            nc.gpsimd.memset(eps_c[:], EPS)
            rcpD_c = persist.tile([128, 1], F32)
            nc.gpsimd.memset(rcpD_c[:], 1.0 / D)
            scl_c = persist.tile([128, 1], F32)
            nc.gpsimd.memset(scl_c[:], SCALE)
            qT = persist.tile([128, H, ROWS], BF16)      # q transposed

            # ---- phase 1+2: load xT, projections ----
            with (
                tc.tile_pool(name="p1", bufs=2, side="right") as p1,
                tc.tile_pool(name="xTp", bufs=1) as xTp,
                tc.tile_pool(name="gpool", bufs=1) as gpool,
                tc.tile_pool(name="bpool", bufs=2) as bpool,
                tc.tile_pool(name="wpool", bufs=2) as wpool,
                tc.tile_pool(name="rows_pool", bufs=1) as rows_pool,
                tc.tile_pool(name="nr", bufs=2) as nr,
                tc.tile_pool(name="ps_tp", bufs=2, space="PSUM") as ps_tp,
                tc.tile_pool(name="ps_acc", bufs=1, space="PSUM") as ps_acc,
            ):
                cs_sb = gpool.tile([ST, 2, NST, HD], BF16, tag="cs")
                nc.sync.dma_start(out=cs_sb[:, 0],
                                  in_=cosf.rearrange("(t p) f -> p t f", p=ST))
                nc.sync.dma_start(out=cs_sb[:, 1],
                                  in_=sinf.rearrange("(t p) f -> p t f", p=ST))
                gq_b = gpool.tile([128, D], BF16, tag="gq")
                nc.sync.dma_start(out=gq_b[:], in_=gq[None, :].partition_broadcast(128))
                gk_b = gpool.tile([128, D], BF16, tag="gk")
                nc.sync.dma_start(out=gk_b[:], in_=gk[None, :].partition_broadcast(128))

                xT = xTp.tile([128, NKC, ROWS], BF16)
                nc.sync.dma_start(out=xT[:],
                                  in_=xT_own.rearrange("(k p) s -> p k s", p=128))

                zpad = p1.tile([128, NKC // 2, 4], BF16, tag="zpad")
                nc.gpsimd.memset(zpad[:], 0.0)
                zv = p1.tile([4, DH], BF16, tag="zv")
                nc.gpsimd.memset(zv[:], 0.0)
                for base in (252, 508):
                    for kb in (kT_bA, kT_bB):
                        nc.sync.dma_start(
                            out=kb[:, base:base + 4]
                            .rearrange("(k p) s -> p k s", p=128),
                            in_=zpad[:])
                    for vb in (v_bA, v_bB):
                        nc.sync.dma_start(out=vb[base:base + 4, :], in_=zv[:])

                def bias_slice(b, cb):
                    t = bpool.tile([1, CB], BF16, tag="bias")
                    nc.sync.dma_start(out=t[:], in_=b[None, cb * CB:(cb + 1) * CB])
                    return t

                # --- pass A: h = x@W + b -> rows_all (scalar engine copies,
                # keeping the vector queue free for rope); sumsq -> var_p ---
                def pass_a(W, bias, rows_all, var_p):
                    for cb in range(NCB):
                        wt = wpool.tile([128, NKC, CB], BF16, tag="w")
                        nc.sync.dma_start(
                            out=wt[:],
                            in_=W[:, cb * CB:(cb + 1) * CB]
                            .rearrange("(k p) c -> p k c", p=128))
                        for st in range(NST):
                            acc = ps_acc.tile([ST, CB], F32, tag=f"acc{st % 2}",
                                              padded_shape=[ST, 512])
                            for kc in range(NKC):
                                nc.tensor.matmul(
                                    acc[:],
                                    xT[:, kc, st * ST:(st + 1) * ST],
                                    wt[:, kc, :],
                                    start=(kc == 0), stop=False)
                            nc.tensor.matmul(
                                acc[:], ones_bf[0:1, 0:ST],
                                bias_slice(bias, cb)[:],
                                start=False, stop=True)
                            sq = nr.tile([ST, CB], F32, tag="sq")
                            nc.scalar.activation(
                                sq[:], acc[:],
                                mybir.ActivationFunctionType.Square,
                                accum_out=var_p[:, st, cb:cb + 1])
                            nc.vector.tensor_copy(
                                rows_all[:, st, cb * CB:(cb + 1) * CB], acc[:])

                # --- v: no norm; 384-col blocks into half bounces ---
                CBV = 384
                def project_v():
                    for cb in range(4):
                        wt = wpool.tile([128, NKC, CBV], BF16, tag="wv")
                        nc.sync.dma_start(
                            out=wt[:],
                            in_=Wv[:, cb * CBV:(cb + 1) * CBV]
                            .rearrange("(k p) c -> p k c", p=128))
                        vb = v_bA if cb < 2 else v_bB
                        c0 = (cb % 2) * CBV
                        for st in range(NST):
                            acc = ps_acc.tile([ST, CBV], F32,
                                              tag=f"accv{st % 2}",
                                              padded_shape=[ST, 512])
                            for kc in range(NKC):
                                nc.tensor.matmul(
                                    acc[:],
                                    xT[:, kc, st * ST:(st + 1) * ST],
                                    wt[:, kc, :],
                                    start=(kc == 0), stop=False)
                            bt = bpool.tile([1, CBV], BF16, tag="biasv")
                            nc.sync.dma_start(
                                out=bt[:],
                                in_=bv[None, cb * CBV:(cb + 1) * CBV])
                            nc.tensor.matmul(
                                acc[:], ones_bf[0:1, 0:ST], bt[:],
                                start=False, stop=True)
                            vr = p1.tile([ST, CBV], BF16, tag="vrow")
                            nc.scalar.activation(vr[:], acc[:],
                                                 mybir.ActivationFunctionType.Copy)
                            rs0 = st * ST + (4 if st >= 2 else 0)
                            nc.sync.dma_start(
                                out=vb[rs0:rs0 + ST, c0:c0 + CBV],
                                in_=vr[:])

                # --- pass B: rstd, normalize (bf16), gain, rope, transpose ---
                def pass_b(rows_all, var_p, g_b, sink):
                    for st in range(NST):
                        var = nr.tile([ST, 1], F32, tag="var")
                        nc.vector.reduce_sum(var[:], var_p[:, st, :],
                                             axis=mybir.AxisListType.X)
                        sdt = nr.tile([ST, 1], F32, tag="sd")
                        nc.scalar.activation(
                            sdt[:], var[:], mybir.ActivationFunctionType.Sqrt,
                            bias=eps_c[0:ST, :], scale=rcpD_c[0:ST, :])
                        rstd = nr.tile([ST, 1], F32, tag="rstd")
                        nc.vector.reciprocal(rstd[:], sdt[:])
                        rows = nr.tile([ST, D], BF16, tag="rows")
                        nc.scalar.activation(
                            rows[:], rows_all[:, st, :],
                            mybir.ActivationFunctionType.Copy,
                            scale=rstd[:])
                        nc.vector.tensor_mul(rows[:], rows[:], g_b[0:ST, :])
                        # rope: out = rows*cosF + swap(rows)*sinF
                        swap = nr.tile([ST, D], BF16, tag="swap")
                        r3 = rows[:].rearrange("p (h r two) -> p h r two",
                                               two=2, r=ROT)
                        s3 = swap[:].rearrange("p (h r two) -> p h r two",
                                               two=2, r=ROT)
                        nc.vector.tensor_copy(s3[:, :, :, 0], r3[:, :, :, 1])
                        nc.vector.tensor_copy(s3[:, :, :, 1], r3[:, :, :, 0])
                        cosb = (cs_sb[:, 0, st, :].unsqueeze(1)
                                .broadcast_to((ST, H, HD)))
                        sinb = (cs_sb[:, 1, st, :].unsqueeze(1)
                                .broadcast_to((ST, H, HD)))
                        rr = rows[:].rearrange("p (h f) -> p h f", h=H)
                        sr = swap[:].rearrange("p (h f) -> p h f", h=H)
                        nc.vector.tensor_mul(rr[:], rr[:], cosb)
                        nc.vector.tensor_mul(sr[:], sr[:], sinb)
                        nc.vector.tensor_add(rows[:], rows[:], swap[:])
                        for kc in range(NKC):
                            pt = ps_tp.tile([128, ST], BF16, tag="tp")
                            nc.tensor.transpose(
                                pt[:], rows[:, kc * 128:(kc + 1) * 128],
                                ident_bf[0:ST, 0:ST])
                            sink(kc, st, pt)

                def q_sink(kc, st, pt):
                    nc.vector.tensor_copy(
                        qT[:, kc, st * ST:(st + 1) * ST], pt[:])

                kstage = {}

                def k_sink(kc, st, pt):
                    if st not in kstage:
                        kstage[st] = p1.tile([128, NKC, ST], BF16, tag="ktr",
                                             name=f"kstage{st}", bufs=4)
                    kt_sb = kstage[st]
                    nc.vector.tensor_copy(kt_sb[:, kc, :], pt[:])
                    if kc == NKC - 1:
                        cs0 = st * ST + (4 if st >= 2 else 0)
                        nc.sync.dma_start(
                            out=kT_bA[:, cs0:cs0 + ST]
                            .rearrange("(k p) s -> p k s", p=128),
                            in_=kt_sb[:, 0:NKC // 2, :])
                        nc.sync.dma_start(
                            out=kT_bB[:, cs0:cs0 + ST]
                            .rearrange("(k p) s -> p k s", p=128),
                            in_=kt_sb[:, NKC // 2:, :])
                        del kstage[st]

                rows_k = rows_pool.tile([ST, NST, D], F32, tag="rows_k")
                var_k = rows_pool.tile([ST, NST, NCB], F32, tag="var_k")
                rows_q = rows_pool.tile([ST, NST, D], F32, tag="rows_q")
                var_q = rows_pool.tile([ST, NST, NCB], F32, tag="var_q")

                # emission order keeps the tensor queue dense; the four
                # half-gathers are interleaved K_A, V_A, K_B, V_B on the
                # serial CC stream so attention (which consumes head 0
                # first) starts after the first two.
                def gather(src_t, dst_t):
                    nc.gpsimd.collective_compute(
                        "AllGather", mybir.AluOpType.bypass,
                        replica_groups=[list(range(NC))],
                        ins=[src_t[:].opt()], outs=[dst_t[:].opt()])

                pass_a(Wk, bk, rows_k, var_k)
                project_v()
                pass_b(rows_k, var_k, gk_b, k_sink)
                gather(kT_bA, kT_gA)
                gather(v_bA, v_gA)
                gather(kT_bB, kT_gB)
                gather(v_bB, v_gB)
                pass_a(Wq, bq, rows_q, var_q)
                pass_b(rows_q, var_q, gq_b, q_sink)

            # ---- phase 4: attention ----
            with tc.tile_pool(name="late", bufs=1) as late:
              attnT = late.tile([128, H, ROWS], BF16)    # attention out^T
              with (
                tc.tile_pool(name="att", bufs=2) as att,
                tc.tile_pool(name="mk", bufs=1) as mk,
                tc.tile_pool(name="psO", bufs=2, space="PSUM") as psO,
                tc.tile_pool(name="ps_s", bufs=2, space="PSUM") as ps_s_pool,
            ):
                mask_sb = mk.tile([KT, max(N_MASKS, 1), CH], BF16)
                nc.sync.dma_start(out=mask_sb[:],
                                  in_=masks.rearrange("n p q -> p n q"))

                # den/PV for block (h, qb) are emitted one block late so
                # the tensor queue never waits on exp: it always has the
                # next block's QK stream in front of den/PV whose exp
                # results are long ready.  This keeps the PE busy enough
                # to hold the HAM clock at 2.4 GHz.
                def flush(h, qb, ps_o, ps_den, expgs, v_h):
                    ntile = BND[qb]
                    for g0, gn, expg in expgs:
                        for i in range(gn):
                            kt = g0 + i
                            src, coff = kt_src(kt)
                            vidx = src * 4 + (coff // KT)
                            first = (kt == 0)
                            last = (kt == ntile - 1)
                            nc.tensor.matmul(
                                ps_den[:], ones_bf[0:KT, :],
                                expg[:, i, :],
                                start=first, stop=last)
                            nc.tensor.matmul(
                                ps_o[:], v_h[:, vidx, :],
                                expg[:, i, :],
                                start=first, stop=last)
                    rd = att.tile([128, CH], F32, tag="rd")
                    nc.vector.reciprocal(rd[:], ps_den[:])
                    nc.vector.tensor_mul(
                        attnT[:, h, qb * CH:(qb + 1) * CH],
                        ps_o[:], rd[:])

                pend = None
                for h in range(H):
                    kg, vg, hh = ((kT_gA, v_gA, h) if h < 6
                                  else (kT_gB, v_gB, h - 6))
                    kT_h = att.tile([128, NC, RPAD], BF16, tag="kTh")
                    nc.sync.dma_start(
                        out=kT_h[:],
                        in_=kg[:, hh * HD:(hh + 1) * HD, :]
                        .rearrange("s p c -> p s c"))
                    v_h = att.tile([KT, NC * 4, HD], BF16, tag="vh")
                    nc.sync.dma_start(
                        out=v_h[:],
                        in_=vg[:, :, hh * HD:(hh + 1) * HD]
                        .rearrange("s (r p) d -> p (s r) d", p=KT))
                    for qb in range(2):
                        ps_o = psO.tile([128, CH], F32, tag="o",
                                        padded_shape=[128, 512])
                        ps_den = psO.tile([128, CH], F32, tag="den",
                                          padded_shape=[128, 512])
                        expgs = []
                        for g0, gn in _groups(qb):
                            # inner dim padded to 256 floats: a matmul output
                            # must not cross a 2KB PSUM bank boundary
                            ps_s = ps_s_pool.tile([KT, EG, CH], F32, tag="s",
                                                  padded_shape=[KT, EG, 256])
                            for i in range(gn):
                                kt = g0 + i
                                src, coff = kt_src(kt)
                                nc.tensor.matmul(
                                    ps_s[:, i, 0:CH],
                                    kT_h[:, src, coff:coff + KT],
                                    qT[:, h, qb * CH:(qb + 1) * CH],
                                    start=True, stop=True)
                            expg = att.tile([KT, EG, CH], BF16, tag="expg",
                                            bufs=18)
                            nc.scalar.activation(
                                expg[:, 0:gn, :], ps_s[:, 0:gn, 0:CH],
                                mybir.ActivationFunctionType.Exp,
                                scale=scl_c[0:KT, :])
                            slot = MASK_GROUP.get((qb, g0))
                            if slot is not None:
                                nc.vector.tensor_mul(
                                    expg[:, 0:gn, :], expg[:, 0:gn, :],
                                    mask_sb[:, slot:slot + gn, :])
                            expgs.append((g0, gn, expg))
                        if pend is not None:
                            flush(*pend)
                        pend = (h, qb, ps_o, ps_den, expgs, v_h)
                flush(*pend)

              # ---- phase 5: output projection ----
              with (
                  tc.tile_pool(name="op", bufs=3) as op,
                  tc.tile_pool(name="ps_op", bufs=2, space="PSUM") as ps_op,
              ):
                  for eb in range(NCB):
                      wt = op.tile([128, NKC, CB], BF16, tag="wo")
                      nc.sync.dma_start(
                          out=wt[:],
                          in_=Wo[:, eb * CB:(eb + 1) * CB]
                          .rearrange("(k p) c -> p k c", p=128))
                      for st in range(NST):
                          acc = ps_op.tile([ST, CB], F32, tag=f"acc{st % 2}")
                          for ct in range(NKC):
                              nc.tensor.matmul(
                                  acc[:],
                                  attnT[:, ct, st * ST:(st + 1) * ST],
                                  wt[:, ct, :],
                                  start=(ct == 0), stop=False)
                          bo_t = op.tile([1, CB], BF16, tag="bo")
                          nc.sync.dma_start(
                              out=bo_t[:], in_=bo[None, eb * CB:(eb + 1) * CB])
                          nc.tensor.matmul(
                              acc[:], ones_bf[0:1, 0:ST], bo_t[:],
                              start=False, stop=True)
                          ot = op.tile([ST, CB], F32, tag="orow")
                          nc.scalar.activation(ot[:], acc[:],
                                               mybir.ActivationFunctionType.Copy)
                          nc.sync.dma_start(
                              out=out_own[st * ST:(st + 1) * ST,
                                          eb * CB:(eb + 1) * CB],
                              in_=ot[:])

    fix_waits(nc)
    return nc


# ---------------- host side ----------------

def _rope_cos_sin():
    """Static index maps for rope angle rows; returns function of freqs."""
    t_dim = ROT - 2 * (ROT // 3)   # 22
    s1 = ROT // 3                  # 21
    idx = np.arange(S)
    f_idx = idx // FRAME
    h_idx = (idx // W_GRID) % H_GRID
    w_idx = idx % W_GRID
    def build(freqs):
        ang = np.empty((S, ROT), np.float32)
        ang[:, :t_dim] = freqs[f_idx, :t_dim]
        ang[:, t_dim:t_dim + s1] = freqs[h_idx, t_dim:t_dim + s1]
        ang[:, t_dim + s1:] = freqs[w_idx, t_dim + s1:]
        cos = np.cos(ang).astype(np.float32)
        sin = np.sin(ang).astype(np.float32)
        cosf = np.repeat(cos, 2, axis=1)                     # [S, 128]
        sinf = np.empty((S, HD), np.float32)
        sinf[:, 0::2] = -sin
        sinf[:, 1::2] = sin
        return cosf, sinf
    return build

_build_cos_sin = _rope_cos_sin()


def _bf16():
    import ml_dtypes
    return ml_dtypes.bfloat16


def _host_masks(c):
    """Per-tile mask slots [N_MASKS, KT, CH] for core c (1.0 allowed).
    Slots are contiguous per exp group; clean tiles get all-ones; pad
    rows of the padded kv layout are always masked off."""
    bf = _bf16()
    out = np.ones((max(N_MASKS, 1), KT, CH), np.float32)
    for (qb, g0), slot in MASK_GROUP.items():
        j = core_chunks(c)[qb]
        q0, q1 = chunk_rows(j)
        qf = (np.arange(q0, q1) // FRAME)                     # [252]
        gn = min(EG, BND[qb] - g0)
        for i in range(gn):
            kt = g0 + i
            if _tile_clean(qb, kt):
                continue                                      # stays ones
            p = np.arange(KT * kt, KT * (kt + 1))             # padded rows
            jj, off = p // 256, p % 256
            real = off < 252
            g = jj * 252 + np.minimum(off, 251)
            kf = g // FRAME
            out[slot + i] = real[:, None] & (kf[:, None] <= qf[None, :])
    return out.astype(bf)


_CACHE = {}


def _get_program():
    if "nc" not in _CACHE:
        _CACHE["nc"] = build_program()
    return _CACHE["nc"]


def build_in_maps(inputs):
    bf = _bf16()
    x = np.asarray(inputs["x"], np.float32)       # [1, S, D]
    freqs = np.asarray(inputs["freqs"], np.float32)
    cosf, sinf = _build_cos_sin(freqs)

    common = {}
    for k in ("Wq", "Wk", "Wv", "Wo", "bq", "bk", "bv", "bo", "gq", "gk"):
        common[k] = np.ascontiguousarray(np.asarray(inputs[k], np.float32).astype(bf))

    in_maps = []
    for c in range(NC):
        ja, jb = core_chunks(c)
        rows = np.concatenate([
            x[0, CH * ja:CH * (ja + 1)], x[0, CH * jb:CH * (jb + 1)]])
        cs = np.concatenate([
            cosf[CH * ja:CH * (ja + 1)], cosf[CH * jb:CH * (jb + 1)]])
        sn = np.concatenate([
            sinf[CH * ja:CH * (ja + 1)], sinf[CH * jb:CH * (jb + 1)]])
        m = {"xT_own": np.ascontiguousarray(rows.T.astype(bf)),
             "cosf": np.ascontiguousarray(cs.astype(bf)),
             "sinf": np.ascontiguousarray(sn.astype(bf)),
             "masks": _host_masks(c)}
        m.update(common)
        in_maps.append(m)
    return in_maps


def kernel(**inputs):
    from concourse.bass_utils import run_bass_kernel_spmd

    in_maps = build_in_maps(inputs)
    nc = _get_program()
    res = run_bass_kernel_spmd(nc, in_maps, list(range(NC)))

    out = np.empty((1, S, D), np.float32)
    for c in range(NC):
        ja, jb = core_chunks(c)
        r = res.results[c]["out_own"]
        out[0, CH * ja:CH * (ja + 1)] = r[:CH]
        out[0, CH * jb:CH * (jb + 1)] = r[CH:]
    return out
